# revision 40
# baseline (speedup 1.0000x reference)
"""Trainium2 kernel for nn_HadamardRotation: y = x @ H, H = 4096x4096 Walsh-Hadamard.

Strategy
--------
H4096 = H64 (x) H64 (Kronecker). Writing d = 64*hi + lo, e = 64*hi' + lo':

    y[r, e] = sum_{hi,lo} H64[lo,lo'] * H64[hi,hi'] * x[r, d]

Two matmul stages with 128-wide contraction (block-diagonal I2 (x) H64
weights), separated by an on-chip "corner turn" (SBUF->SBUF DMA partition
shuffle), all operating in the transposed domain (d on partitions, rows on
the free axis).

The corner turn dominates; it is tuned for the DMA engines' per-descriptor
cost: the whole per-core row range (L=2048) is kept in one SBUF-resident
intermediate so every turn descriptor is a full 4KB line, and the 32 turn
DMAs are spread over multiple queues so their descriptors hit all 16 DMA
engines. Input and output DRAM tensors exactly mirror the SBUF tiles
(contiguous 4-8KB per partition per DMA).

FLOPs: 2 * 128/4096 of the naive matmul = 16x reduction.

Data parallel over 8 cores: rows sharded 16384 -> 8 x 2048, weights
replicated. Host does the layout permutes / final f32 cast (not timed).

Per-core layouts (R = 2048 rows = L, N = 512 matmul slab):
  xt DRAM in  [16, 128, IB*L]: xt[g, q, j*L+rr] = x[rr, 128*(IB*g+j)+q]
  B1 (128,128): B1[64*mu+lo, 32*(2nu+mu)+c] = H64[lo, 2c+nu]
  B2 (128,128): B2[64*nu+32*mu+a, 2*hi'+nu] = H64[2*a+mu, hi']
  stage A (chunk a): u[m, a, rr] = sum_k B1[k, m] xg[k, j, rr]
      => u[32*(2nu+mu) + c, a] holds (hi = 2a+mu, lo' = 2c+nu)
  corner turn (chunk c): vc[32t+a, rr] = u[32t+c, a, rr]
  stage B (chunk c): yb[m2, rr] = sum_q B2[q, m2] vc[q, rr]
      => yb[2*hi'+nu] = y[rr, 64*hi' + 2*c + nu]  (bf16)
  Y DRAM out [32/OB, 128, OB*L]: mirrors the SBUF yb tiles; host
     unscrambles + casts to f32.
"""

import math
import numpy as np
import ml_dtypes

import concourse.bass as bass
import concourse.mybir as mybir
import concourse.tile as tile
from concourse import bacc
from concourse.bass_utils import run_bass_kernel_spmd

N_CORES = 8
DIM = 4096
R_TOTAL = 4 * 4096          # rows after flattening (4, 4096, DIM)
R = R_TOTAL // N_CORES      # rows per core
L = R                       # all rows resident: 4KB turn descriptors
N = 512                     # matmul free-dim slab (one PSUM bank of fp32)
TS = L // N                 # matmul slabs per chunk
MODE = "bf16"               # storage dtype for x/intermediate/output

# tuning knobs
CFG = dict(
    IB=2,              # chunks per input DMA / xg tile
    OB=2,              # chunks per output DMA / yb tile
    OB2=2,             # chunks per output DMA in the deferred ts=0 pass
    ucopy_engs="vector,scalar",  # stage-A psum->sbuf copies (DVE is free of
                                 # stream work during the whole A phase now)
    ycopy_engs="scalar",         # main-B copies: ACT only (DVE streams then)
    y2copy_engs="vector,scalar",  # deferred-pass copies
    in_engs="sync",
    out_engs="sync",
    out2_engs="gpsimd",
    turn_engs="gpsimd,sync",     # queues that never carry blocked out-DMAs
    W_DVE=512,         # leading r-window turned on the DVE stream unit
                       # (takes 2x its bytes off the saturated DMA fabric)
    turn_splits=4,     # stream-transpose instrs (finer DVE interleave)
    xbufs=2, vbufs=6, ybufs=2, psA=4, psB=4,
)


def _walsh_hadamard64():
    h = np.array([[1.0]], dtype=np.float64)
    while h.shape[0] < 64:
        h = np.block([[h, h], [h, -h]]) / math.sqrt(2.0)
    return h.astype(np.float32)


def _build_weights(H64):
    # B1[64*mu+lo, 32*(2*nu+mu')+c] = H64[lo, 2c+nu] if mu'==mu else 0
    B1 = np.zeros((128, 128), dtype=np.float32)
    b1v = B1.reshape(2, 64, 2, 2, 32)       # [mu, lo, nu, mu', c]
    for mu in range(2):
        for nu in range(2):
            b1v[mu, :, nu, mu, :] = H64[:, nu::2]
    # B2[64*nu+32*mu+a, 2*hi'+nu'] = H64[2a+mu, hi'] if nu'==nu else 0
    B2 = np.zeros((128, 128), dtype=np.float32)
    b2v = B2.reshape(2, 2, 32, 64, 2)       # [nu, mu, a, hi', nu']
    for nu in range(2):
        for mu in range(2):
            b2v[nu, mu, :, :, nu] = H64[mu::2, :]
    return B1, B2


_NC_CACHE = {}


def _build_bass(cfg=None):
    cfg = dict(CFG, **(cfg or {}))
    key = tuple(sorted(cfg.items()))
    if key in _NC_CACHE:
        return _NC_CACHE[key]

    f32 = mybir.dt.float32
    bf16 = mybir.dt.bfloat16

    IB, OB, OB2 = cfg["IB"], cfg["OB"], cfg["OB2"]
    NG = 32 // IB               # input chunk groups
    NCB = 32 // OB              # output chunk batches (main, ts>=1 slabs)
    NCB2 = 32 // OB2            # output chunk batches (deferred ts=0 pass)
    WD = cfg["W_DVE"]
    LM = L - WD                 # r-range covered by the main (DMA-turn) pass
    TSM = LM // N               # slabs in the main pass

    nc = bacc.Bacc("TRN2", target_bir_lowering=False, debug=False,
                   num_devices=N_CORES)
    LH = L // 2                 # r-half loaded per input DMA (ts-pair-major)
    xt_d = nc.dram_tensor("xt", [2 * NG, 128, IB * LH], bf16,
                          kind="ExternalInput")
    B1_d = nc.dram_tensor("B1", [128, 128], bf16, kind="ExternalInput")
    B2_d = nc.dram_tensor("B2", [128, 128], bf16, kind="ExternalInput")
    Y_d = nc.dram_tensor("Y", [NCB, 128, OB * LM], bf16, kind="ExternalOutput")
    Y2_d = nc.dram_tensor("Y2", [NCB2, 128, OB2 * WD], bf16,
                          kind="ExternalOutput")

    with tile.TileContext(nc) as tc:
        with (
            tc.tile_pool(name="wpool", bufs=1) as wpool,
            tc.tile_pool(name="xpool", bufs=cfg["xbufs"]) as xpool,
            tc.tile_pool(name="upool", bufs=1) as upool,
            tc.tile_pool(name="vpool", bufs=cfg["vbufs"]) as vpool,
            tc.tile_pool(name="vdpool", bufs=1) as vdpool,
            tc.tile_pool(name="ypool", bufs=cfg["ybufs"]) as ypool,
            tc.tile_pool(name="psA", bufs=cfg["psA"], space="PSUM") as psA,
            tc.tile_pool(name="psB", bufs=cfg["psB"], space="PSUM") as psB,
        ):
            B1_sb = wpool.tile([128, 128], bf16)
            nc.sync.dma_start(B1_sb[:], B1_d[:])
            B2_sb = wpool.tile([128, 128], bf16)
            nc.sync.dma_start(B2_sb[:], B2_d[:])



            def eng_list(names):
                return [getattr(nc, nm.strip()) for nm in names.split(",")]

            ucopy_engs = eng_list(cfg["ucopy_engs"])
            ycopy_engs = eng_list(cfg["ycopy_engs"])
            y2copy_engs = eng_list(cfg["y2copy_engs"])
            turn_engs = eng_list(cfg["turn_engs"])
            out_engs = eng_list(cfg["out_engs"])
            out2_engs = eng_list(cfg["out2_engs"])
            in_engs = eng_list(cfg["in_engs"])

            def copy(engs, i, dst, src):
                e = engs[i % len(engs)]
                if e is nc.scalar:
                    nc.scalar.copy(dst, src)
                else:
                    e.tensor_copy(dst, src)

            u_all = upool.tile([128, 32, L], bf16)
            ut = u_all.tensor
            PU = u_all.ap[0][0]  # partition stride in elements

            TSPL = cfg["turn_splits"]
            v_dve = None
            if WD:
                v_dve = vdpool.tile([128, 32, WD], bf16, name="v_dve")

            def emit_stream_turn():
                # v[32t+a, c, rr] = u[32t+c, a, rr] via DVE 32x32 stream
                # transposes over the leading W_DVE r-window, all chunks.
                vt = v_dve.tensor
                PV = v_dve.ap[0][0]
                WS = WD // TSPL
                for s in range(TSPL):
                    in_ap = bass.AP(ut, s * WS,
                                    [[PU, 128], [1, WS], [L, 32]])
                    out_ap = bass.AP(vt, s * WS,
                                     [[PV, 128], [1, WS], [WD, 32]])
                    nc.vector.transpose(out_ap, in_ap)

            # stage A: r-half-major so the DVE stream turn (which only needs
            # the first slab) can start at mid-phase
            for th in range(2):
                for g in range(NG):
                    xg = xpool.tile([128, IB, LH], bf16)
                    gg = th * NG + g
                    in_engs[gg % len(in_engs)].dma_start(xg[:], xt_d[gg, :, :])
                    for j in range(IB):
                        a = IB * g + j
                        for tl in range(TS // 2):
                            ts = th * (TS // 2) + tl
                            pu = psA.tile([128, N], f32)
                            nc.tensor.matmul(pu[:], B1_sb[:],
                                             xg[:, j, tl * N:(tl + 1) * N],
                                             start=True, stop=True)
                            copy(ucopy_engs, a * TS + ts,
                                 u_all[:, a, ts * N:(ts + 1) * N], pu[:])
            # stream turn emitted AFTER the full A phase: it occupies the DVE
            # queue for ~76us, which now overlaps the main (DMA-turn) B pass
            # instead of starving the A-phase copies.
            if WD:
                emit_stream_turn()

            # main B pass: DMA corner turn + the ts>=1 slabs
            for cb in range(NCB):
                yb = ypool.tile([128, OB, LM], bf16)
                for j in range(OB):
                    c = cb * OB + j
                    vc = vpool.tile([128, LM], bf16)
                    in_ap = bass.AP(ut, c * PU + WD,
                                    [[32 * PU, 4], [L, 32], [1, LM]])
                    turn_engs[c % len(turn_engs)].dma_start(vc[:], in_ap)
                    for tm in range(TSM):
                        py = psB.tile([128, N], f32)
                        nc.tensor.matmul(py[:], B2_sb[:],
                                         vc[:, tm * N:(tm + 1) * N],
                                         start=True, stop=True)
                        copy(ycopy_engs, c * TSM + tm,
                             yb[:, j, tm * N:(tm + 1) * N], py[:])
                out_engs[cb % len(out_engs)].dma_start(Y_d[cb, :, :], yb[:])

            # deferred pass: the ts=0 slab from the stream-turned v_dve
            if WD:
                for cb in range(NCB2):
                    yb2 = ypool.tile([128, OB2, WD], bf16, name="yb2")
                    for j in range(OB2):
                        c = cb * OB2 + j
                        py = psB.tile([128, N], f32)
                        nc.tensor.matmul(py[:], B2_sb[:], v_dve[:, c, :],
                                         start=True, stop=True)
                        copy(y2copy_engs, c, yb2[:, j, :], py[:])
                    out2_engs[cb % len(out2_engs)].dma_start(
                        Y2_d[cb, :, :], yb2[:])

    nc.compile()
    _NC_CACHE[key] = nc
    return nc


def _prep_inputs(x, H, cfg=None):
    cfg = dict(CFG, **(cfg or {}))
    IB = cfg["IB"]
    NG = 32 // IB
    H64 = (np.asarray(H, dtype=np.float32)[::64, ::64] * 8.0).astype(np.float32)
    B1, B2 = _build_weights(H64)
    B1 = B1.astype(ml_dtypes.bfloat16)
    B2 = B2.astype(ml_dtypes.bfloat16)
    xf = np.asarray(x, dtype=np.float32).reshape(R_TOTAL, DIM)
    in_maps = []
    LH = L // 2
    for i in range(N_CORES):
        shard = xf[i * R:(i + 1) * R]                     # (R, DIM)
        # [(th, rr), a, q] -> [th, g, q, j, rr]
        xt = shard.reshape(2, LH, 32, 128).transpose(0, 2, 3, 1)  # [th,a,q,rr]
        xt = xt.reshape(2, NG, IB, 128, LH).transpose(0, 1, 3, 2, 4)
        xt = np.ascontiguousarray(xt, dtype=ml_dtypes.bfloat16)
        xt = xt.reshape(2 * NG, 128, IB * LH)
        in_maps.append({"xt": xt, "B1": B1, "B2": B2})
    return in_maps


def _unscramble(results, cfg=None):
    cfg = dict(CFG, **(cfg or {}))
    OB, OB2, WD = cfg["OB"], cfg["OB2"], cfg["W_DVE"]
    NCB, NCB2 = 32 // OB, 32 // OB2
    LM = L - WD
    outs = []
    for i in range(N_CORES):
        # [cb, (hi', nu), j, rr] -> [rr, hi', (cb, j, nu)]
        Y = results[i]["Y"]      # [NCB, 128, OB*LM] bf16, rows WD..L
        ym = np.asarray(Y, dtype=np.float32).reshape(NCB, 64, 2, OB, LM)
        ym = ym.transpose(4, 1, 0, 3, 2).reshape(LM, DIM)
        if WD:
            Y2 = results[i]["Y2"]    # [NCB2, 128, OB2*WD] bf16, rows 0..WD
            y2 = np.asarray(Y2, dtype=np.float32).reshape(NCB2, 64, 2, OB2, WD)
            y2 = y2.transpose(4, 1, 0, 3, 2).reshape(WD, DIM)
            y = np.concatenate([y2, ym], axis=0)
        else:
            y = ym
        outs.append(y)
    return np.concatenate(outs, axis=0).reshape(4, 4096, DIM).astype(np.float32)


def kernel(x, H, _trace=False, _cfg=None):
    nc = _build_bass(_cfg)
    in_maps = _prep_inputs(x, H, _cfg)
    res = run_bass_kernel_spmd(nc, in_maps, core_ids=list(range(N_CORES)),
                               trace=_trace)
    out = _unscramble(res.results, _cfg)
    if _trace:
        return out, res
    return out
